# revision 13
# baseline (speedup 1.0000x reference)
"""Fused multi-head attention (QKV + RoPE2D + softmax + out-proj) on 8 TRN2 cores.

Sharding: batch-parallel. B == n_cores == 8, so each core runs one batch
element end-to-end; weights are replicated. No collectives needed.

Per-core dataflow (matmul operands in bf16, accumulation in f32 PSUM):
  qkT[D,n] = (x @ w_{q,k}.T).T   (lhsT = w chunks, rhs = xT tiles)
  RoPE on qkT: rot = stream_shuffle(qkT) (lane permute i^16 per 32-block),
  qk' = qkT*cos2 + rot*sin2      (signs folded into sin2 host-side; all bf16)
  v[n,dd] = x @ w_v.T            (v-proj groups streamed inside the feeder)
  attention per head-PAIR t (heads A=2t on partitions 0:64 of qk', B=2t+1
  on 64:128), one flat software pipeline over (t, ch, jt):
     sT[:, 0:512]   = kA-tile.T @ qA-chunk   (K=64, PE rows 0-63)
     sT[:, 512:1024]= kB-tile.T @ qB-chunk   (K=64, PE rows 64-127)
     -> issued back-to-back, the two row-tiles stream CONCURRENTLY
     ex = exp(sT/8) on ACT, one [128,1024] call for both heads
     av_X += [v_X | ones].T @ ex[:, X-half]  (K=128, accumulated over jt)
  The av matmuls trail the exp by PEND_DEPTH pipeline steps ACROSS ch and
  pair boundaries, so the in-order PE queue never blocks on the ACT engine.
  Softmax sums ride in av row 64; per-(pair,ch) normalization (DVE
  reciprocal -> gpsimd partition_broadcast -> gpsimd multiply) is emitted
  one pair late so its chain is off the critical path.  Pair 6/7 norms are
  scheduled explicitly inside pair-7's attention window, which also pulls
  out-projection prefill matmuls (the window is ACT-bound, so the PE has
  slack there); the remaining projection groups stream right after with
  their ct7 column gated only on the already-landed norms.
  y = outT.T @ w_projT (+bias via DVE add) -> DMA out.

The next pair's QKV matmuls and the v-projection groups are interleaved
into the current pair's attention (generator-based software pipelining)
so the in-order PE queue always has ready work during exp waits.
All bulk inputs are host-packed into their exact SBUF layouts so every
input DMA is a contiguous >=2KB-per-partition transfer.  The prologue's
critical transfers are issued alternately from the Sync and Activation
sequencers (both HWDGE-capable) so descriptor generation is not
serialized on one engine; the final out DMAs alternate the same way.
"""

import os
import numpy as np

B, N, C = 8, 1024, 1024
H, HD = 16, 64
P = 128
NT = N // P          # 8 n-tiles
CT = C // P          # 8 c-tiles
TP = H // 2          # 8 head-pairs (qk D-tiles per q/k)
SCALE = HD ** -0.5   # 1/8

_CACHE = {}

SHUF_MASK = [i ^ 16 for i in range(32)]   # rotate_half as 32-lane permute
BCAST_MASK = [0] * 32                     # broadcast lane 0 of each 32-block


def _build_nc():
    import concourse.mybir as mybir
    from concourse import bacc, tile
    from contextlib import ExitStack

    f32 = mybir.dt.float32
    bf16 = mybir.dt.bfloat16
    EXP = mybir.ActivationFunctionType.Exp

    nc = bacc.Bacc(
        "TRN2", target_bir_lowering=False, debug=False,
        enable_asserts=False, num_devices=B,
    )

    xt_d = nc.dram_tensor("xt", [P, 2, CT, 512], bf16, kind="ExternalInput")
    cos2_d = nc.dram_tensor("cos2", [P, N], bf16, kind="ExternalInput")
    sin2_d = nc.dram_tensor("sin2", [P, N], bf16, kind="ExternalInput")
    wqk_d = nc.dram_tensor("wqk", [TP, 2, P, CT * P], bf16, kind="ExternalInput")
    wv_d = nc.dram_tensor("wv", [P, 2, CT, 512], bf16, kind="ExternalInput")
    wp_d = nc.dram_tensor("wp", [P, CT, C], bf16, kind="ExternalInput")
    bias_d = nc.dram_tensor("bias", [1, C], f32, kind="ExternalInput")
    # output in bf16: halves the tail DMA; the host upcasts to f32
    out_d = nc.dram_tensor("out", [N, C], bf16, kind="ExternalOutput")

    with tile.TileContext(nc) as tc, ExitStack() as ctx:
        const = ctx.enter_context(tc.tile_pool(name="const", bufs=1))
        vpool = ctx.enter_context(tc.tile_pool(name="vpool", bufs=1))
        otpool = ctx.enter_context(tc.tile_pool(name="otpool", bufs=1))
        qkpre = ctx.enter_context(tc.tile_pool(name="qkpre", bufs=2))
        qkfin = ctx.enter_context(tc.tile_pool(name="qkfin", bufs=4))
        expp = ctx.enter_context(tc.tile_pool(name="expp", bufs=7))
        nrmp = ctx.enter_context(tc.tile_pool(name="nrmp", bufs=2))
        ybp = ctx.enter_context(tc.tile_pool(name="ybp", bufs=3))
        sT_ps = ctx.enter_context(tc.tile_pool(name="sT_ps", bufs=2, space="PSUM"))
        av_ps = ctx.enter_context(tc.tile_pool(name="av_ps", bufs=2, space="PSUM"))
        mi_ps = ctx.enter_context(tc.tile_pool(name="mi_ps", bufs=2, space="PSUM"))

        # ---- input DMA: the prologue-critical transfers zig-zag between
        # the Sync and Activation sequencers (each DMA_DIRECT2D costs
        # ~650ns of sequencer time, so one engine alone serializes the
        # head); every transfer is contiguous per partition ----
        wqk = const.tile([P, TP, 2, CT, P], bf16)  # all q,k weight D-tiles
        cos2 = const.tile([P, N], bf16)
        sin2 = const.tile([P, N], bf16)
        xt = const.tile([P, 2, CT, 512], bf16)     # [p, ch, ct, n-within]
        wv = const.tile([P, 2, CT, 512], bf16)     # [p, ch-of-dd, ct, dd]

        def wqk_half(t, s, h):
            hs = slice(h * 4, h * 4 + 4)
            return (wqk[:, t, s, hs].rearrange("p ct c -> p (ct c)"),
                    wqk_d[t, s, :, h * 512:h * 512 + 512])

        prologue = [
            # (sync-op, scalar-op) pairs, issued alternately
            (wqk_half(0, 0, 0), wqk_half(0, 0, 1)),
            ((xt[:, 0, 0:2], xt_d[:, 0, 0:2]), (xt[:, 0, 2:4], xt_d[:, 0, 2:4])),
            ((xt[:, 0, 4:6], xt_d[:, 0, 4:6]), (xt[:, 0, 6:8], xt_d[:, 0, 6:8])),
            (wqk_half(0, 1, 0), wqk_half(0, 1, 1)),
            ((cos2[:, 0:512], cos2_d[:, 0:512]), (sin2[:, 0:512], sin2_d[:, 0:512])),
            ((wv[:, 0, 0:4], wv_d[:, 0, 0:4]), (wv[:, 0, 4:8], wv_d[:, 0, 4:8])),
            ((xt[:, 1, 0:2], xt_d[:, 1, 0:2]), (xt[:, 1, 2:4], xt_d[:, 1, 2:4])),
            ((xt[:, 1, 4:6], xt_d[:, 1, 4:6]), (xt[:, 1, 6:8], xt_d[:, 1, 6:8])),
            ((cos2[:, 512:N], cos2_d[:, 512:N]), (sin2[:, 512:N], sin2_d[:, 512:N])),
            ((wqk[:, 1, 0].rearrange("p ct c -> p (ct c)"), wqk_d[1, 0]),
             (wqk[:, 1, 1].rearrange("p ct c -> p (ct c)"), wqk_d[1, 1])),
        ]
        for (sdst, ssrc), (adst, asrc) in prologue:
            nc.sync.dma_start(sdst, ssrc)
            nc.scalar.dma_start(adst, asrc)

        wp = const.tile([P, CT, C], bf16)
        bias_bc = const.tile([P, C], f32)
        nc.sync.dma_start(bias_bc[:1, :], bias_d[:])
        nc.gpsimd.partition_broadcast(bias_bc[:], bias_bc[:1, :])

        def xts(ct, nt):
            """xT tile [128 c-local, 128 n] for (ct, n-tile nt)."""
            return xt[:, nt // 4, ct, (nt % 4) * P:(nt % 4 + 1) * P]

        # v storage: [128 j-local, NT j-tiles, H heads x (64 v + 1 ones col)]
        v_all = vpool.tile([P, NT, H * (HD + 1)], bf16)
        ones_c = const.tile([P, H], f32)
        nc.vector.memset(ones_c[:], 1.0)
        for jt in range(NT):
            nc.vector.tensor_copy(
                v_all[:, jt, :].rearrange("p (h c) -> p h c", c=HD + 1)[:, :, HD:],
                ones_c[:].rearrange("p (h o) -> p h o", o=1))
        # out.T accumulator: [128 c-local, CT c-tiles, 1024 n]
        outT = otpool.tile([P, CT, N], bf16)
        # normalization staging: reciprocal rows land on partitions
        # 0/32/64/96 via DMA; the rest must be initialized for the
        # broadcast shuffle's read
        st_bc = otpool.tile([P, N], f32)
        nc.vector.memset(st_bc[:], 1.0)

        qk_fin = {}
        spairs = {}
        v_ready = set()

        def prepare_pair(t):
            """Generator: qk D-tile matmuls + RoPE for pair t, yielding after
            each PE instruction. ch-chunked so the ch0 halves of q,k (and
            hence the first scores) never wait on ch1 inputs."""
            fins = [qkfin.tile([P, N], bf16, tag="fin", name=f"fin{s}")
                    for s in range(2)]
            qk_fin[t] = fins
            pres = [qkpre.tile([P, N], bf16, tag=f"pre{s}", name=f"pre{s}")
                    for s in range(2)]
            for ch in range(2):
                cs = slice(ch * 512, (ch + 1) * 512)
                for s in range(2):  # 0 = q, 1 = k
                    qps = mi_ps.tile([P, 512], f32, tag="mi", name="qps")
                    for ct in range(CT):
                        nc.tensor.matmul(
                            qps[:],
                            wqk[:, t, s, ct],
                            xt[:, ch, ct, :],
                            start=(ct == 0), stop=(ct == CT - 1),
                        )
                        yield
                    nc.vector.tensor_copy(pres[s][:, cs], qps[:])
                for s in range(2):
                    rot = qkpre.tile([P, 512], bf16, tag="rot", name="rot")
                    nc.vector.stream_shuffle(rot[:], pres[s][:, cs], SHUF_MASK)
                    tmp = qkpre.tile([P, 512], bf16, tag="tmp", name="tmp")
                    nc.vector.tensor_mul(tmp[:], pres[s][:, cs], cos2[:, cs])
                    nc.vector.tensor_mul(fins[s][:, cs], rot[:], sin2[:, cs])
                    nc.vector.tensor_add(fins[s][:, cs], fins[s][:, cs], tmp[:])
                    yield

        def v_group(nt, ch):
            """Generator: one v-projection group (8 accumulating matmuls +
            copy into the packed v_all layout)."""
            vps = mi_ps.tile([P, 512], f32, tag="mi", name="vps")
            for ct in range(CT):
                nc.tensor.matmul(
                    vps[:],
                    xts(ct, nt),
                    wv[:, ch, ct, :],
                    start=(ct == 0), stop=(ct == CT - 1),
                )
                yield
            nc.vector.tensor_copy(
                v_all[:, nt, :].rearrange(
                    "p (h c) -> p h c", c=HD + 1)[:, 8 * ch:8 * ch + 8, :HD],
                vps[:])
            v_ready.add((nt, ch))

        def chain(*gens):
            for g in gens:
                yield from g

        feeder = None

        def pull(k):
            nonlocal feeder
            if feeder is None:
                return
            for _ in range(k):
                if next(feeder, "done") == "done":
                    feeder = None
                    return

        def ensure_v(nt, vch):
            """Drain the feeder until v_group(nt, vch) has been emitted."""
            while (nt, vch) not in v_ready:
                assert feeder is not None, f"v_group({nt},{vch}) unreachable"
                pull(1)

        # ---- the flat attention pipeline ----
        # av matmuls trail their exp by PEND_DEPTH pipeline steps so the
        # in-order PE queue never blocks on the ACT engine, even across
        # ch/pair boundaries
        PEND_DEPTH = 1
        PEND_DEPTH_P0 = 6   # pair 0: defer avs past the input-DMA window
        pend_q = []   # [(t, ch, jt, ex, avA, avB), ...] awaiting av matmuls

        def emit_pend():
            """Emit the oldest trailing av matmuls; on a ch-pass close, also
            emit the epilogue copies and (for pairs <= 6, one pair late) the
            previous pair's normalization."""
            if not pend_q:
                return
            t, ch, jt, ex, avA, avB = pend_q.pop(0)
            hA, hB = 2 * t, 2 * t + 1
            ensure_v(jt, t // 4)
            nc.tensor.matmul(
                avA[:], v_all[:, jt, hA * (HD + 1):(hA + 1) * (HD + 1)],
                ex[:, 0:512], start=(jt == 0), stop=(jt == NT - 1))
            nc.tensor.matmul(
                avB[:], v_all[:, jt, hB * (HD + 1):(hB + 1) * (HD + 1)],
                ex[:, 512:1024], start=(jt == 0), stop=(jt == NT - 1))
            if jt == NT - 1:
                cs = slice(ch * 512, (ch + 1) * 512)
                spair = spairs[t]
                # rows 0:64 = unnormalized out.T; row 64 = softmax sums,
                # staged into spair[0, head*N + ch*512 : ...] by DVE copies.
                # The sum copies go FIRST so a normalization chain emitted
                # right after this close starts ~1.4us earlier on the DVE.
                nc.vector.tensor_copy(
                    spair[0:1, ch * 512:ch * 512 + 512], avA[HD:HD + 1, :])
                nc.vector.tensor_copy(
                    spair[0:1, N + ch * 512:N + ch * 512 + 512],
                    avB[HD:HD + 1, :])
                nc.vector.tensor_copy(outT[0:64, t, cs], avA[:HD, :])
                nc.vector.tensor_copy(outT[64:128, t, cs], avB[:HD, :])
                if t >= 1 and t <= 6 and ch == 0:
                    # previous pair's sums have had a full ch-pass to land;
                    # pair-6/7 norms are scheduled inside attention(6)/(7)
                    normalize_gp(t - 1, None)

        def normalize_gp(t, ch):
            """Normalize outT[:, t, cs] (cs = full N when ch is None): DVE
            reciprocals of the two head sum rows, DMA-staged to partitions
            0/32/64/96 of st_bc, one stream_shuffle broadcast within each
            32-block, and the multiply on the otherwise-idle GPSIMD."""
            spair = spairs[t]
            cs = slice(0, N) if ch is None else slice(ch * 512, (ch + 1) * 512)
            w = cs.stop - cs.start
            rcp = nrmp.tile([1, 2 * N], f32, tag="rcp", name="rcp")
            # spair layout: [A ch0 | A ch1 | B ch0 | B ch1], so head A's cs
            # slice is at cs.start and head B's at N + cs.start
            nc.vector.reciprocal_approx_fast(
                rcp[0:1, 0:w], spair[0:1, cs.start:cs.start + w])
            nc.vector.reciprocal_approx_fast(
                rcp[0:1, N:N + w], spair[0:1, N + cs.start:N + cs.start + w])
            for q, r in ((0, 0), (32, 0), (64, N), (96, N)):
                nc.sync.dma_start(st_bc[q:q + 1, cs], rcp[0:1, r:r + w])
            rb = nrmp.tile([P, N], f32, tag="rb", name="rb")
            nc.vector.stream_shuffle(rb[:, cs], st_bc[:, cs], BCAST_MASK)
            nc.gpsimd.tensor_mul(outT[:, t, cs], outT[:, t, cs], rb[:, cs])

        def attention(t, npull, sched=None):
            qf, kf = qk_fin.pop(t)
            spairs[t] = nrmp.tile([1, 2 * N], f32, tag="spair", name="spair")
            depth = PEND_DEPTH_P0 if t == 0 else PEND_DEPTH
            step = 0
            for ch in range(2):
                cs = slice(ch * 512, (ch + 1) * 512)
                avA = av_ps.tile([HD + 1, 512], f32, tag="av", name="avA")
                avB = av_ps.tile([HD + 1, 512], f32, tag="av", name="avB")
                for jt in range(NT):
                    if t == 0:
                        # stream jt0-3 scores+exps during the input-DMA wait;
                        # pp0-ch1 pulls start at step 2 (its xt-ch1 inputs
                        # land ~14us with the two-engine prologue, before
                        # these matmuls reach the head of the PE queue)
                        pull({0: 0, 1: 0, 2: 4, 3: 8, 4: 6}.get(step, npull))
                    sT = sT_ps.tile([P, N], f32, tag="sT", name="sT")
                    js = slice(jt * P, (jt + 1) * P)
                    # two K=64 row-tiles, back-to-back -> concurrent on PE
                    nc.tensor.matmul(sT[:, 0:512], kf[0:64, js], qf[0:64, cs],
                                     start=True, stop=True)
                    nc.tensor.matmul(sT[:, 512:1024], kf[64:128, js],
                                     qf[64:128, cs], start=True, stop=True)
                    ex = expp.tile([P, N], bf16, tag="ex", name="ex")
                    nc.scalar.activation(ex[:], sT[:], EXP, scale=SCALE)
                    if sched is not None:
                        sched(step)
                    elif t != 0:
                        pull(npull)
                    while len(pend_q) >= depth:
                        emit_pend()
                    pend_q.append((t, ch, jt, ex, avA, avB))
                    step += 1

        open_g = {}

        def proj_group(nt, ch2):
            """Generator: emit the out-proj accumulation matmuls for group
            (nt, ch2) into a fresh mi PSUM buf, yielding after each; the
            buf is stashed in open_g for the close."""
            yps = mi_ps.tile([P, 512], f32, tag="mi", name="yps")
            open_g[(nt, ch2)] = yps
            es = slice(ch2 * 512, (ch2 + 1) * 512)
            ns = slice(nt * P, (nt + 1) * P)
            for ct in range(CT):
                nc.tensor.matmul(
                    yps[:], outT[:, ct, ns], wp[:, ct, es],
                    start=(ct == 0), stop=(ct == CT - 1),
                )
                yield

        close_n = [0]

        def proj_close(nt, ch2):
            """Bias add + out DMA for a finished group; DMA issue alternates
            between the Sync and Activation sequencers."""
            yps = open_g.pop((nt, ch2))
            es = slice(ch2 * 512, (ch2 + 1) * 512)
            ns = slice(nt * P, (nt + 1) * P)
            yb = ybp.tile([P, 512], bf16, tag="yb", name="yb")
            nc.vector.tensor_add(yb[:], yps[:], bias_bc[:, es])
            if close_n[0] % 2 == 0:
                nc.sync.dma_start(out_d[ns, es], yb[:])
            else:
                nc.scalar.dma_start(out_d[ns, es], yb[:])
            close_n[0] += 1

        # ---- prologue: pair-0 qk ch0-half only; the rest feeds pair 0 ----
        pp0 = prepare_pair(0)
        feeder = pp0
        pull(18)   # 16 ch0 matmuls + 2 RoPE-chunk yields

        # bulk weight DMAs are gated behind pair-0's first RoPE output via a
        # dummy WAW write, so they don't steal front DMA bandwidth from the
        # critical xt/wv0 transfers (queues transfer concurrently, so pure
        # emission order does not prioritize)
        gate_src = qk_fin[0][0]
        nc.vector.tensor_copy(wv[0:1, 1, 0, 0:1], gate_src[0:1, 0:1])
        nc.sync.dma_start(wv[:, 1], wv_d[:, 1])
        for t in range(2, TP):
            for s in range(2):
                nc.vector.tensor_copy(wqk[0:1, t, s, 0, 0:1],
                                      gate_src[0:1, 0:1])
                nc.sync.dma_start(
                    wqk[:, t, s].rearrange("p ct c -> p (ct c)"), wqk_d[t, s])
        nc.vector.tensor_copy(wp[0:1, 0, 0:1], gate_src[0:1, 0:1])
        nc.sync.dma_start(wp[:], wp_d[:])

        # feeder work for each pair's attention window
        feeders = [
            chain(pp0, v_group(0, 0), v_group(1, 0), v_group(2, 0),
                  v_group(3, 0), v_group(4, 0), v_group(5, 0),
                  v_group(6, 0), v_group(7, 0), prepare_pair(1)),
            chain(v_group(0, 1), v_group(1, 1), v_group(2, 1), prepare_pair(2)),
            chain(v_group(3, 1), v_group(4, 1), v_group(5, 1), prepare_pair(3)),
            chain(v_group(6, 1), v_group(7, 1), prepare_pair(4)),
            prepare_pair(5),
            prepare_pair(6),
            prepare_pair(7),
            None,
        ]
        npulls = [6, 4, 4, 4, 2, 2, 2, 2]

        def p6_sched(step):
            pull(npulls[6])
            if step == 10:
                # pair-6 ch0 sums closed at step ~8; landing the ch0-half
                # norm here lets pair-7's projection prefill take ct6 early
                normalize_gp(6, 0)

        for t in range(TP - 1):
            feeder = feeders[t] if feeders[t] is not None else feeder
            attention(t, npulls[t], sched=p6_sched if t == 6 else None)
            pull(10 ** 6)

        # ---- pair 7: attention window doubles as projection prefill ----
        # The window is ACT(exp)-paced, so the PE has slack; pull the first
        # two projection groups (nt=0, both es halves) through it, and land
        # the pair-6/7-ch0 norms early so the groups' ct6/ct7 columns are
        # ungated by the time they reach the head of the PE queue.
        g00 = proj_group(0, 0)
        g01 = proj_group(0, 1)

        def pair7_feeder():
            # ct0..6 prefills (14 yields): ct0..5 gated on pairs 0-5 norms,
            # ct6 on the pair-6 ch0 norm that landed at the end of pair 6
            for s in range(7):
                next(g00)
                yield
            for s in range(7):
                next(g01)
                yield

        p7f = pair7_feeder()

        def p7_sched(step):
            if step == 1:
                normalize_gp(6, 1)
            if 2 <= step <= 8:
                next(p7f, None)
                next(p7f, None)
            if step == 9:
                normalize_gp(7, 0)
            if step == 15:
                # ct7 closes of the prefilled groups: their pair-7 ch0 norm
                # (emitted step 9) lands while the last scores stream, and
                # these fill the PE while the final exp drains
                next(g00, None)
                next(g01, None)

        attention(TP - 1, 0, sched=p7_sched)
        for _ in range(2):
            next(p7f, None)    # safety drain (no-op when exhausted)
        next(g00, None)
        next(g01, None)
        while pend_q:          # flush trailing avs + epilogues
            emit_pend()
        normalize_gp(7, 1)
        proj_close(0, 0)
        proj_close(0, 1)

        # ---- remaining out-proj groups: nt1-3 need only the ch0-half norms
        # (landed mid-pair-7); nt4-7's ct7 gate (pair-7 ch1 norm) is covered
        # by the preceding groups' ~12us of matmuls ----
        for nt in range(1, NT):
            for ch2 in range(2):
                g = proj_group(nt, ch2)
                for _ in range(CT):
                    next(g)
                proj_close(nt, ch2)

    nc.compile()
    return nc


def get_nc():
    if "nc" not in _CACHE:
        _CACHE["nc"] = _build_nc()
    return _CACHE["nc"]


def _host_inputs(x, xpos, w_qkv, w_proj, b_proj):
    """Host-side reshapes: transposes, RoPE tables, weight packing into the
    exact SBUF layouts (so device DMAs are contiguous)."""
    import ml_dtypes

    x = np.asarray(x, dtype=np.float32)
    xpos = np.asarray(xpos)
    w_qkv = np.asarray(w_qkv, dtype=np.float32)
    w_proj = np.asarray(w_proj, dtype=np.float32)
    b_proj = np.asarray(b_proj, dtype=np.float32).reshape(1, C)

    # xt[b] = [p, ch, ct, n-within]: xT[ct*128+p, ch*512+n]
    xT = x.transpose(0, 2, 1)                      # [B, C, N]
    xt = np.ascontiguousarray(
        xT.reshape(B, CT, P, 2, 512).transpose(0, 2, 3, 1, 4))

    # RoPE tables in [d, n] orientation, two head-copies stacked to 128 rows.
    inv_freq = (100.0 ** (-np.arange(16, dtype=np.float64) / 16.0))
    py = xpos[..., 0].astype(np.float64)  # [B, N]
    px = xpos[..., 1].astype(np.float64)
    angy = py[:, :, None] * inv_freq      # [B, N, 16]
    angx = px[:, :, None] * inv_freq
    cos64 = np.concatenate(
        [np.cos(angy), np.cos(angy), np.cos(angx), np.cos(angx)], axis=2)
    sin64 = np.concatenate(
        [-np.sin(angy), np.sin(angy), -np.sin(angx), np.sin(angx)], axis=2)
    cos2 = np.ascontiguousarray(
        np.tile(cos64, (1, 1, 2)).transpose(0, 2, 1)).astype(np.float32)
    sin2 = np.ascontiguousarray(
        np.tile(sin64, (1, 1, 2)).transpose(0, 2, 1)).astype(np.float32)

    # wqk[t, s] = [c-local partition, ct*P + d] — exact SBUF layout
    wqk = np.zeros((TP, 2, P, CT * P), dtype=np.float32)
    for t in range(TP):
        for s in range(2):
            rows = w_qkv[s * C + t * P:s * C + (t + 1) * P, :]  # [P(d), C]
            wqk[t, s] = rows.reshape(P, CT, P).transpose(2, 1, 0).reshape(
                P, CT * P)
    # wv = [c-local partition, ch-of-dd, ct, dd-within]
    wv = np.ascontiguousarray(
        w_qkv[2 * C:3 * C, :].T.reshape(CT, P, 2, 512).transpose(1, 2, 0, 3))
    # wp = [c-local partition, ct, e]
    wp = np.ascontiguousarray(w_proj.T.reshape(CT, P, C).transpose(1, 0, 2))

    def mcast(a):
        return np.ascontiguousarray(a).astype(ml_dtypes.bfloat16)

    shared = dict(wqk=mcast(wqk), wv=mcast(wv), wp=mcast(wp), bias=b_proj)
    in_maps = []
    for b in range(B):
        m = dict(shared)
        m["xt"] = mcast(xt[b])
        m["cos2"] = mcast(cos2[b])
        m["sin2"] = mcast(sin2[b])
        in_maps.append(m)
    return in_maps


def kernel(x, xpos, w_qkv, w_proj, b_proj):
    from concourse import bass_utils

    nc = get_nc()
    in_maps = _host_inputs(x, xpos, w_qkv, w_proj, b_proj)
    res = bass_utils.run_bass_kernel_spmd(
        nc, in_maps, core_ids=list(range(B)),
        trace=bool(int(os.environ.get("BASS_ATTN_TRACE", "0"))),
    )
    out = np.stack([np.asarray(res.results[b]["out"]) for b in range(B)],
                   axis=0).astype(np.float32)
    _CACHE["last_results"] = res
    return out


# revision 20
# speedup vs baseline: 1.0343x; 1.0343x over previous
"""Fused multi-head attention (QKV + RoPE2D + softmax + out-proj) on 8 TRN2 cores.

Sharding: batch-parallel. B == n_cores == 8, so each core runs one batch
element end-to-end; weights are replicated. No collectives needed.

Per-core dataflow (matmul operands in bf16, accumulation in f32 PSUM):
  qkT[D,n] = (x @ w_{q,k}.T).T   (lhsT = w chunks, rhs = xT tiles)
  RoPE on qkT: rot = stream_shuffle(qkT) (lane permute i^16 per 32-block),
  qk' = qkT*cos2 + rot*sin2      (signs folded into sin2 host-side; all bf16)
  v[n,dd] = x @ w_v.T            (v-proj groups streamed inside the feeder)
  attention per head-PAIR t (heads A=2t on partitions 0:64 of qk', B=2t+1
  on 64:128), one flat software pipeline over (t, ch, jt):
     sT[:, 0:512]   = kA-tile.T @ qA-chunk   (K=64, PE rows 0-63)
     sT[:, 512:1024]= kB-tile.T @ qB-chunk   (K=64, PE rows 64-127)
     -> issued back-to-back, the two row-tiles stream CONCURRENTLY
     ex = exp(sT/8) on ACT, one [128,1024] call for both heads
     av_X += [v_X | ones].T @ ex[:, X-half]  (K=128, accumulated over jt)
  The av matmuls trail the exp by PEND_DEPTH pipeline steps ACROSS ch and
  pair boundaries, so the in-order PE queue never blocks on the ACT engine.
  Softmax sums ride in av row 64; per-(pair,ch) normalization (DVE
  reciprocal -> gpsimd partition_broadcast -> gpsimd multiply) is emitted
  one pair late so its chain is off the critical path.  Pair 6/7 norms are
  scheduled explicitly inside pair-7's attention window, which also pulls
  out-projection prefill matmuls (the window is ACT-bound, so the PE has
  slack there); the remaining projection groups stream right after with
  their ct7 column gated only on the already-landed norms.
  y = outT.T @ w_projT (+bias via DVE add) -> DMA out.

The next pair's QKV matmuls and the v-projection groups are interleaved
into the current pair's attention (generator-based software pipelining)
so the in-order PE queue always has ready work during exp waits.
All bulk inputs are host-packed into their exact SBUF layouts so every
input DMA is a contiguous >=2KB-per-partition transfer.  The prologue's
critical transfers are issued alternately from the Sync and Activation
sequencers (both HWDGE-capable) so descriptor generation is not
serialized on one engine; the final out DMAs alternate the same way.
"""

import os
import numpy as np

B, N, C = 8, 1024, 1024
H, HD = 16, 64
P = 128
NT = N // P          # 8 n-tiles
CT = C // P          # 8 c-tiles
TP = H // 2          # 8 head-pairs (qk D-tiles per q/k)
SCALE = HD ** -0.5   # 1/8

_CACHE = {}

SHUF_MASK = [i ^ 16 for i in range(32)]   # rotate_half as 32-lane permute
BCAST_MASK = [0] * 32                     # broadcast lane 0 of each 32-block


def _build_nc():
    import concourse.mybir as mybir
    from concourse import bacc, tile
    from contextlib import ExitStack

    f32 = mybir.dt.float32
    bf16 = mybir.dt.bfloat16
    EXP = mybir.ActivationFunctionType.Exp

    nc = bacc.Bacc(
        "TRN2", target_bir_lowering=False, debug=False,
        enable_asserts=False, num_devices=B,
    )

    xt_d = nc.dram_tensor("xt", [P, 2, CT, 512], bf16, kind="ExternalInput")
    cos2_d = nc.dram_tensor("cos2", [P, N], bf16, kind="ExternalInput")
    sin2_d = nc.dram_tensor("sin2", [P, N], bf16, kind="ExternalInput")
    wqk_d = nc.dram_tensor("wqk", [TP, 2, P, CT * P], bf16, kind="ExternalInput")
    wv_d = nc.dram_tensor("wv", [P, 2, CT, 512], bf16, kind="ExternalInput")
    wp_d = nc.dram_tensor("wp", [P, CT, C], bf16, kind="ExternalInput")
    bias_d = nc.dram_tensor("bias", [1, C], f32, kind="ExternalInput")
    # output in bf16: halves the tail DMA; the host upcasts to f32
    out_d = nc.dram_tensor("out", [N, C], bf16, kind="ExternalOutput")

    with tile.TileContext(nc) as tc, ExitStack() as ctx:
        const = ctx.enter_context(tc.tile_pool(name="const", bufs=1))
        vpool = ctx.enter_context(tc.tile_pool(name="vpool", bufs=1))
        otpool = ctx.enter_context(tc.tile_pool(name="otpool", bufs=1))
        qkpre = ctx.enter_context(tc.tile_pool(name="qkpre", bufs=2))
        qkfin = ctx.enter_context(tc.tile_pool(name="qkfin", bufs=4))
        expp = ctx.enter_context(tc.tile_pool(name="expp", bufs=7))
        nrmp = ctx.enter_context(tc.tile_pool(name="nrmp", bufs=2))
        ybp = ctx.enter_context(tc.tile_pool(name="ybp", bufs=3))
        sT_ps = ctx.enter_context(tc.tile_pool(name="sT_ps", bufs=2, space="PSUM"))
        av_ps = ctx.enter_context(tc.tile_pool(name="av_ps", bufs=2, space="PSUM"))
        mi_ps = ctx.enter_context(tc.tile_pool(name="mi_ps", bufs=2, space="PSUM"))

        # ---- input DMA: the prologue-critical transfers zig-zag between
        # the Sync and Activation sequencers (each DMA_DIRECT2D costs
        # ~650ns of sequencer time, so one engine alone serializes the
        # head); every transfer is contiguous per partition ----
        wqk = const.tile([P, TP, 2, CT, P], bf16)  # all q,k weight D-tiles
        cos2 = const.tile([P, N], bf16)
        sin2 = const.tile([P, N], bf16)
        xt = const.tile([P, 2, CT, 512], bf16)     # [p, ch, ct, n-within]
        wv = const.tile([P, 2, CT, 512], bf16)     # [p, ch-of-dd, ct, dd]

        # larger transfers beat the ~4-deep per-engine DMA semaphore ring
        # (transfer n+4 waits for transfer n to fully complete)
        sync_head = [
            (wqk[:, 0, 0].rearrange("p ct c -> p (ct c)"), wqk_d[0, 0]),
            (xt[:, 0, 0:4], xt_d[:, 0, 0:4]),
            (wqk[:, 0, 1].rearrange("p ct c -> p (ct c)"), wqk_d[0, 1]),
            (xt[:, 1, 0:4], xt_d[:, 1, 0:4]),
            (wqk[:, 1, 0].rearrange("p ct c -> p (ct c)"), wqk_d[1, 0]),
        ]
        scalar_head = [
            (xt[:, 0, 4:8], xt_d[:, 0, 4:8]),
            (cos2[:, 0:512], cos2_d[:, 0:512]),
            (sin2[:, 0:512], sin2_d[:, 0:512]),
            (xt[:, 1, 4:8], xt_d[:, 1, 4:8]),
            (wv[:, 0], wv_d[:, 0]),
            (cos2[:, 512:N], cos2_d[:, 512:N]),
            (sin2[:, 512:N], sin2_d[:, 512:N]),
            (wqk[:, 1, 1].rearrange("p ct c -> p (ct c)"), wqk_d[1, 1]),
        ]
        for i in range(max(len(sync_head), len(scalar_head))):
            if i < len(sync_head):
                nc.sync.dma_start(*sync_head[i])
            if i < len(scalar_head):
                nc.scalar.dma_start(*scalar_head[i])

        wp = const.tile([P, CT, C], bf16)
        bias_bc = const.tile([P, C], f32)
        nc.sync.dma_start(bias_bc[:1, :], bias_d[:])
        nc.gpsimd.partition_broadcast(bias_bc[:], bias_bc[:1, :])

        def xts(ct, nt):
            """xT tile [128 c-local, 128 n] for (ct, n-tile nt)."""
            return xt[:, nt // 4, ct, (nt % 4) * P:(nt % 4 + 1) * P]

        # v storage: [128 j-local, NT j-tiles, H heads x (64 v + 1 ones col)]
        v_all = vpool.tile([P, NT, H * (HD + 1)], bf16)
        ones_c = const.tile([P, H], f32)
        nc.vector.memset(ones_c[:], 1.0)
        for jt in range(NT):
            nc.vector.tensor_copy(
                v_all[:, jt, :].rearrange("p (h c) -> p h c", c=HD + 1)[:, :, HD:],
                ones_c[:].rearrange("p (h o) -> p h o", o=1))
        # out.T accumulator: [128 c-local, CT c-tiles, 1024 n]
        outT = otpool.tile([P, CT, N], bf16)
        # normalization staging: reciprocal rows land on partitions
        # 0/32/64/96 via DMA; the rest must be initialized for the
        # broadcast shuffle's read
        st_bc = otpool.tile([P, N], f32)
        nc.vector.memset(st_bc[:], 1.0)

        qk_fin = {}
        spairs = {}
        v_ready = set()

        def prepare_pair(t):
            """Generator: qk D-tile matmuls + RoPE for pair t, yielding after
            each PE instruction. ch-chunked so the ch0 halves of q,k (and
            hence the first scores) never wait on ch1 inputs."""
            fins = [qkfin.tile([P, N], bf16, tag="fin", name=f"fin{s}")
                    for s in range(2)]
            qk_fin[t] = fins
            pres = [qkpre.tile([P, N], bf16, tag=f"pre{s}", name=f"pre{s}")
                    for s in range(2)]
            for ch in range(2):
                cs = slice(ch * 512, (ch + 1) * 512)
                for s in range(2):  # 0 = q, 1 = k
                    qps = mi_ps.tile([P, 512], f32, tag="mi", name="qps")
                    for ct in range(CT):
                        nc.tensor.matmul(
                            qps[:],
                            wqk[:, t, s, ct],
                            xt[:, ch, ct, :],
                            start=(ct == 0), stop=(ct == CT - 1),
                        )
                        yield
                    nc.vector.tensor_copy(pres[s][:, cs], qps[:])
                for s in range(2):
                    rot = qkpre.tile([P, 512], bf16, tag="rot", name="rot")
                    nc.vector.stream_shuffle(rot[:], pres[s][:, cs], SHUF_MASK)
                    tmp = qkpre.tile([P, 512], bf16, tag="tmp", name="tmp")
                    nc.vector.tensor_mul(tmp[:], pres[s][:, cs], cos2[:, cs])
                    nc.vector.tensor_mul(fins[s][:, cs], rot[:], sin2[:, cs])
                    nc.vector.tensor_add(fins[s][:, cs], fins[s][:, cs], tmp[:])
                    yield

        def v_group(nt, ch):
            """Generator: one v-projection group (8 accumulating matmuls +
            copy into the packed v_all layout)."""
            vps = mi_ps.tile([P, 512], f32, tag="mi", name="vps")
            for ct in range(CT):
                nc.tensor.matmul(
                    vps[:],
                    xts(ct, nt),
                    wv[:, ch, ct, :],
                    start=(ct == 0), stop=(ct == CT - 1),
                )
                yield
            nc.vector.tensor_copy(
                v_all[:, nt, :].rearrange(
                    "p (h c) -> p h c", c=HD + 1)[:, 8 * ch:8 * ch + 8, :HD],
                vps[:])
            v_ready.add((nt, ch))

        def chain(*gens):
            for g in gens:
                yield from g

        feeder = None

        def pull(k):
            nonlocal feeder
            if feeder is None:
                return
            for _ in range(k):
                if next(feeder, "done") == "done":
                    feeder = None
                    return

        def ensure_v(nt, vch):
            """Drain the feeder until v_group(nt, vch) has been emitted."""
            while (nt, vch) not in v_ready:
                assert feeder is not None, f"v_group({nt},{vch}) unreachable"
                pull(1)

        # ---- the flat attention pipeline ----
        # av matmuls trail their exp by PEND_DEPTH pipeline steps so the
        # in-order PE queue never blocks on the ACT engine, even across
        # ch/pair boundaries
        PEND_DEPTH = 1
        PEND_DEPTH_P0 = 6   # pair 0: defer avs past the input-DMA window
        pend_q = []   # [(t, ch, jt, ex, avA, avB), ...] awaiting av matmuls

        def emit_pend():
            """Emit the oldest trailing av matmuls; on a ch-pass close, also
            emit the epilogue copies and (for pairs <= 6, one pair late) the
            previous pair's normalization."""
            if not pend_q:
                return
            t, ch, jt, ex, avA, avB = pend_q.pop(0)
            hA, hB = 2 * t, 2 * t + 1
            ensure_v(jt, t // 4)
            nc.tensor.matmul(
                avA[:], v_all[:, jt, hA * (HD + 1):(hA + 1) * (HD + 1)],
                ex[:, 0:512], start=(jt == 0), stop=(jt == NT - 1))
            nc.tensor.matmul(
                avB[:], v_all[:, jt, hB * (HD + 1):(hB + 1) * (HD + 1)],
                ex[:, 512:1024], start=(jt == 0), stop=(jt == NT - 1))
            if jt == NT - 1:
                cs = slice(ch * 512, (ch + 1) * 512)
                spair = spairs[t]
                # rows 0:64 = unnormalized out.T; row 64 = softmax sums,
                # staged into spair[0, head*N + ch*512 : ...] by DVE copies.
                # The sum copies go FIRST so a normalization chain emitted
                # right after this close starts ~1.4us earlier on the DVE.
                nc.vector.tensor_copy(
                    spair[0:1, ch * 512:ch * 512 + 512], avA[HD:HD + 1, :])
                nc.vector.tensor_copy(
                    spair[0:1, N + ch * 512:N + ch * 512 + 512],
                    avB[HD:HD + 1, :])
                nc.vector.tensor_copy(outT[0:64, t, cs], avA[:HD, :])
                nc.vector.tensor_copy(outT[64:128, t, cs], avB[:HD, :])
                if t >= 1 and t <= 6 and ch == 0:
                    # previous pair's sums have had a full ch-pass to land;
                    # pair-6/7 norms are scheduled inside attention(6)/(7)
                    normalize_gp(t - 1, None)

        def normalize_gp(t, ch):
            """Normalize outT[:, t, cs] (cs = full N when ch is None): DVE
            reciprocals of the two head sum rows, DMA-staged to partitions
            0/32/64/96 of st_bc, one stream_shuffle broadcast within each
            32-block, and the multiply on the otherwise-idle GPSIMD."""
            spair = spairs[t]
            cs = slice(0, N) if ch is None else slice(ch * 512, (ch + 1) * 512)
            w = cs.stop - cs.start
            rcp = nrmp.tile([1, 2 * N], f32, tag="rcp", name="rcp")
            # spair layout: [A ch0 | A ch1 | B ch0 | B ch1], so head A's cs
            # slice is at cs.start and head B's at N + cs.start
            nc.vector.reciprocal_approx_fast(
                rcp[0:1, 0:w], spair[0:1, cs.start:cs.start + w])
            nc.vector.reciprocal_approx_fast(
                rcp[0:1, N:N + w], spair[0:1, N + cs.start:N + cs.start + w])
            for q, r in ((0, 0), (32, 0), (64, N), (96, N)):
                nc.sync.dma_start(st_bc[q:q + 1, cs], rcp[0:1, r:r + w])
            rb = nrmp.tile([P, N], f32, tag="rb", name="rb")
            nc.vector.stream_shuffle(rb[:, cs], st_bc[:, cs], BCAST_MASK)
            nc.gpsimd.tensor_mul(outT[:, t, cs], outT[:, t, cs], rb[:, cs])

        def attention(t, npull, sched=None):
            qf, kf = qk_fin.pop(t)
            spairs[t] = nrmp.tile([1, 2 * N], f32, tag="spair", name="spair")
            depth = PEND_DEPTH_P0 if t == 0 else PEND_DEPTH
            step = 0
            for ch in range(2):
                cs = slice(ch * 512, (ch + 1) * 512)
                avA = av_ps.tile([HD + 1, 512], f32, tag="av", name="avA")
                avB = av_ps.tile([HD + 1, 512], f32, tag="av", name="avB")
                for jt in range(NT):
                    if t == 0:
                        # stream jt0-3 scores+exps during the input-DMA wait:
                        # no DMA-gated feeder matmuls enter the PE stream
                        # until the jt4 scores need pp0-ch1 emitted
                        pull(0 if step < 4 else (18 if step == 4 else npull))
                    sT = sT_ps.tile([P, N], f32, tag="sT", name="sT")
                    js = slice(jt * P, (jt + 1) * P)
                    # two K=64 row-tiles, back-to-back -> concurrent on PE
                    nc.tensor.matmul(sT[:, 0:512], kf[0:64, js], qf[0:64, cs],
                                     start=True, stop=True)
                    nc.tensor.matmul(sT[:, 512:1024], kf[64:128, js],
                                     qf[64:128, cs], start=True, stop=True)
                    ex = expp.tile([P, N], bf16, tag="ex", name="ex")
                    nc.scalar.activation(ex[:], sT[:], EXP, scale=SCALE)
                    if sched is not None:
                        sched(step)
                    elif t != 0:
                        pull(npull)
                    while len(pend_q) >= depth:
                        emit_pend()
                    pend_q.append((t, ch, jt, ex, avA, avB))
                    step += 1

        open_g = {}

        def proj_group(nt, ch2, yps=None):
            """Generator: emit the out-proj accumulation matmuls for group
            (nt, ch2) into yps (a fresh mi PSUM buf when not given),
            yielding after each; the buf is stashed in open_g for the
            close."""
            if yps is None:
                yps = mi_ps.tile([P, 512], f32, tag="mi", name="yps")
            open_g[(nt, ch2)] = yps
            es = slice(ch2 * 512, (ch2 + 1) * 512)
            ns = slice(nt * P, (nt + 1) * P)
            for ct in range(CT):
                nc.tensor.matmul(
                    yps[:], outT[:, ct, ns], wp[:, ct, es],
                    start=(ct == 0), stop=(ct == CT - 1),
                )
                yield

        close_n = [0]

        def proj_close(nt, ch2):
            """Bias add + out DMA for a finished group; DMA issue alternates
            between the Sync and Activation sequencers."""
            yps = open_g.pop((nt, ch2))
            es = slice(ch2 * 512, (ch2 + 1) * 512)
            ns = slice(nt * P, (nt + 1) * P)
            yb = ybp.tile([P, 512], bf16, tag="yb", name="yb")
            nc.vector.tensor_add(yb[:], yps[:], bias_bc[:, es])
            if close_n[0] % 2 == 0:
                nc.sync.dma_start(out_d[ns, es], yb[:])
            else:
                nc.scalar.dma_start(out_d[ns, es], yb[:])
            close_n[0] += 1

        # ---- prologue: pair-0 qk ch0-half only; the rest feeds pair 0 ----
        pp0 = prepare_pair(0)
        feeder = pp0
        pull(18)   # 16 ch0 matmuls + 2 RoPE-chunk yields

        # bulk weight DMAs are gated behind pair-0's first RoPE output via a
        # dummy WAW write, so they don't steal front DMA bandwidth from the
        # critical xt/wv0 transfers (queues transfer concurrently, so pure
        # emission order does not prioritize)
        # (the WAW dummies run on the otherwise-idle GPSIMD so they don't
        # pollute the DVE queue between the pair-0 RoPE ops)
        gate_src = qk_fin[0][0]
        nc.gpsimd.tensor_copy(wv[0:1, 1, 0, 0:1], gate_src[0:1, 0:1])
        nc.sync.dma_start(wv[:, 1], wv_d[:, 1])
        for t in range(2, TP):
            for s in range(2):
                nc.gpsimd.tensor_copy(wqk[0:1, t, s, 0, 0:1],
                                      gate_src[0:1, 0:1])
                nc.sync.dma_start(
                    wqk[:, t, s].rearrange("p ct c -> p (ct c)"), wqk_d[t, s])
        nc.gpsimd.tensor_copy(wp[0:1, 0, 0:1], gate_src[0:1, 0:1])
        nc.sync.dma_start(wp[:], wp_d[:])

        # feeder work for each pair's attention window
        feeders = [
            chain(pp0, v_group(0, 0), v_group(1, 0), v_group(2, 0),
                  v_group(3, 0), v_group(4, 0), v_group(5, 0),
                  v_group(6, 0), v_group(7, 0), prepare_pair(1)),
            chain(v_group(0, 1), v_group(1, 1), v_group(2, 1), prepare_pair(2)),
            chain(v_group(3, 1), v_group(4, 1), v_group(5, 1), prepare_pair(3)),
            chain(v_group(6, 1), v_group(7, 1), prepare_pair(4)),
            prepare_pair(5),
            prepare_pair(6),
            prepare_pair(7),
            None,
        ]
        npulls = [6, 4, 4, 4, 2, 2, 2, 2]

        for t in range(TP - 1):
            feeder = feeders[t] if feeders[t] is not None else feeder
            attention(t, npulls[t])
            pull(10 ** 6)

        # ---- pair 7: attention window doubles as projection prefill ----
        # The window is ACT(exp)-paced, so the PE has slack; pull the first
        # two projection groups (nt=0, both es halves) through it, and land
        # the pair-6/7-ch0 norms early so the groups' ct6/ct7 columns are
        # ungated by the time they reach the head of the PE queue.
        g00 = proj_group(0, 0)
        g01 = proj_group(0, 1)

        def pair7_feeder():
            # ct0..5 prefills (12 yields, gated only on pairs 0-5 norms),
            # then ct6 (2, gated on the pair-6 norm landed by ~step 7)
            for s in range(6):
                next(g00)
                yield
            for s in range(6):
                next(g01)
                yield
            next(g00)          # ct6
            yield
            next(g01)          # ct6
            yield

        p7f = pair7_feeder()

        def p7_sched(step):
            if step == 1:
                normalize_gp(6, None)
            if 2 <= step <= 7:
                next(p7f, None)
                next(p7f, None)
            if step == 9:
                next(p7f, None)   # g00 ct6 (pair-6 norm landed ~step 7)
                next(p7f, None)   # g01 ct6
            if step == 10:
                normalize_gp(7, 0)
            if step == 15:
                # ct7 closes of the prefilled groups: their pair-7 ch0 norm
                # (emitted step 10) lands while the last scores stream, and
                # these fill the PE while the final exp drains
                next(g00, None)
                next(g01, None)

        attention(TP - 1, 0, sched=p7_sched)
        for _ in range(2):
            next(p7f, None)    # safety drain (no-op when exhausted)
        next(g00, None)
        next(g01, None)
        while pend_q:          # flush trailing avs + epilogues
            emit_pend()
        # closes BEFORE the ch1 norm so their DVE bias-adds (which unblock
        # the mi ring) aren't queued behind the norm reciprocals
        proj_close(0, 0)
        proj_close(0, 1)
        normalize_gp(7, 1)

        # ---- remaining out-proj groups, six in flight: pairs of groups
        # alternate between fresh mi bufs and the attention-phase sT PSUM
        # (free once the last exp has read its scores).  nt1-3 are fully
        # ungated (pair-7 ch0 norm landed mid-window); nt4-7's ct7 gate
        # (the ch1 norm) is covered by the preceding groups' matmuls ----
        order = [(nt, ch2) for nt in range(1, NT) for ch2 in range(2)]
        for gi in range(0, len(order), 2):
            (na, ca), (nb, cb) = order[gi], order[gi + 1]
            if gi % 4 == 0:
                wide = sT_ps.tile([P, N], f32, tag="sT", name="wide")
                ypsa, ypsb = wide[:, 0:512], wide[:, 512:1024]
            else:
                ypsa = ypsb = None
            ga = proj_group(na, ca, ypsa)
            gb = proj_group(nb, cb, ypsb)
            for _ in range(CT):
                next(ga)
            for _ in range(CT):
                next(gb)
            proj_close(na, ca)
            proj_close(nb, cb)

    nc.compile()
    return nc


def get_nc():
    if "nc" not in _CACHE:
        _CACHE["nc"] = _build_nc()
    return _CACHE["nc"]


def _host_inputs(x, xpos, w_qkv, w_proj, b_proj):
    """Host-side reshapes: transposes, RoPE tables, weight packing into the
    exact SBUF layouts (so device DMAs are contiguous)."""
    import ml_dtypes

    x = np.asarray(x, dtype=np.float32)
    xpos = np.asarray(xpos)
    w_qkv = np.asarray(w_qkv, dtype=np.float32)
    w_proj = np.asarray(w_proj, dtype=np.float32)
    b_proj = np.asarray(b_proj, dtype=np.float32).reshape(1, C)

    # xt[b] = [p, ch, ct, n-within]: xT[ct*128+p, ch*512+n]
    xT = x.transpose(0, 2, 1)                      # [B, C, N]
    xt = np.ascontiguousarray(
        xT.reshape(B, CT, P, 2, 512).transpose(0, 2, 3, 1, 4))

    # RoPE tables in [d, n] orientation, two head-copies stacked to 128 rows.
    inv_freq = (100.0 ** (-np.arange(16, dtype=np.float64) / 16.0))
    py = xpos[..., 0].astype(np.float64)  # [B, N]
    px = xpos[..., 1].astype(np.float64)
    angy = py[:, :, None] * inv_freq      # [B, N, 16]
    angx = px[:, :, None] * inv_freq
    cos64 = np.concatenate(
        [np.cos(angy), np.cos(angy), np.cos(angx), np.cos(angx)], axis=2)
    sin64 = np.concatenate(
        [-np.sin(angy), np.sin(angy), -np.sin(angx), np.sin(angx)], axis=2)
    cos2 = np.ascontiguousarray(
        np.tile(cos64, (1, 1, 2)).transpose(0, 2, 1)).astype(np.float32)
    sin2 = np.ascontiguousarray(
        np.tile(sin64, (1, 1, 2)).transpose(0, 2, 1)).astype(np.float32)

    # wqk[t, s] = [c-local partition, ct*P + d] — exact SBUF layout
    wqk = np.zeros((TP, 2, P, CT * P), dtype=np.float32)
    for t in range(TP):
        for s in range(2):
            rows = w_qkv[s * C + t * P:s * C + (t + 1) * P, :]  # [P(d), C]
            wqk[t, s] = rows.reshape(P, CT, P).transpose(2, 1, 0).reshape(
                P, CT * P)
    # wv = [c-local partition, ch-of-dd, ct, dd-within]
    wv = np.ascontiguousarray(
        w_qkv[2 * C:3 * C, :].T.reshape(CT, P, 2, 512).transpose(1, 2, 0, 3))
    # wp = [c-local partition, ct, e]
    wp = np.ascontiguousarray(w_proj.T.reshape(CT, P, C).transpose(1, 0, 2))

    def mcast(a):
        return np.ascontiguousarray(a).astype(ml_dtypes.bfloat16)

    shared = dict(wqk=mcast(wqk), wv=mcast(wv), wp=mcast(wp), bias=b_proj)
    in_maps = []
    for b in range(B):
        m = dict(shared)
        m["xt"] = mcast(xt[b])
        m["cos2"] = mcast(cos2[b])
        m["sin2"] = mcast(sin2[b])
        in_maps.append(m)
    return in_maps


def kernel(x, xpos, w_qkv, w_proj, b_proj):
    from concourse import bass_utils

    nc = get_nc()
    in_maps = _host_inputs(x, xpos, w_qkv, w_proj, b_proj)
    res = bass_utils.run_bass_kernel_spmd(
        nc, in_maps, core_ids=list(range(B)),
        trace=bool(int(os.environ.get("BASS_ATTN_TRACE", "0"))),
    )
    out = np.stack([np.asarray(res.results[b]["out"]) for b in range(B)],
                   axis=0).astype(np.float32)
    _CACHE["last_results"] = res
    return out
